# revision 2
# baseline (speedup 1.0000x reference)
"""CenterLoss Trainium2 kernel (batch-parallel over 8 cores).

Math shortcut: the reference's mask keeps only distmat[b, labels[b]], so the
loss needs just the gathered centers rows, never the 1024x10000 distmat:
  loss = (1/B) * sum_b ||x_b - centers[labels_b]||^2  (+ clip mass of zeros)
The clipped zero entries contribute exactly (C-1)*1e-12 per row; added on
host. The clip on real distances (~1e3) is a no-op.

Per core (128 batch rows), all engines start at t0:
  SP ring:  labels DMA [128,8] i32 (each label replicated x8 -> 32B/partition,
            avoids 4B read-modify-write descriptors), then x DMA [128,512] bf16
  Pool:     16B SWDGE decoy DMA (warms the dynamic-DMA ucode path), then the
            indirect gather centers[labels] with inline f32->bf16 cast once
            labels land
  DVE (hidden under the gather): xx = rowsum(x*x) -> sx[:,1]; xm2 = -2x
  DVE (post-gather): u = c + xm2; s1 = rowsum(c*u) -> sx[:,0]
  PE:       acc[1,2] = ones.T @ sx  (bf16 single pass; partition reduction)
  DVE:      acc -> sbuf;  SP: DMA [1,2] out
Host sums the 8x2 partials (the "all-reduce") and divides by B.

bf16 error budget: each row sum is 512 products with independent rounding
(~2^-9 relative); the aggregate relative error lands ~1e-5, far inside the
2e-2 gate (measured 1.0e-5).
"""

import numpy as np

_BATCH = 1024
_FEAT = 512
_NCLASSES = 10000
_NCORES = 8
_ROWS = _BATCH // _NCORES  # 128
_P = 128
_LABPAD = 8

_state = {}


def _build_nc():
    import concourse.bass as bass
    import concourse.mybir as mybir
    from concourse import bacc

    f32 = mybir.dt.float32
    bf16 = mybir.dt.bfloat16
    i32 = mybir.dt.int32
    nc = bacc.Bacc("TRN2", target_bir_lowering=False, debug=False)
    x_d = nc.dram_tensor("x", [_ROWS, _FEAT], bf16, kind="ExternalInput").ap()
    labels_d = nc.dram_tensor(
        "labels", [_ROWS, _LABPAD], i32, kind="ExternalInput"
    ).ap()
    centers_d = nc.dram_tensor(
        "centers", [_NCLASSES, _FEAT], f32, kind="ExternalInput"
    ).ap()
    out_d = nc.dram_tensor("out", [1, 2], f32, kind="ExternalOutput").ap()

    from contextlib import ExitStack

    with ExitStack() as _es:
        decoy_t = _es.enter_context(nc.sbuf_tensor("decoy_t", [1, 4], f32))
        lab_t = _es.enter_context(nc.sbuf_tensor("lab_t", [_ROWS, _LABPAD], i32))
        x_t = _es.enter_context(nc.sbuf_tensor("x_t", [_P, _FEAT], bf16))
        c_t = _es.enter_context(nc.sbuf_tensor("c_t", [_P, _FEAT], bf16))
        sq_t = _es.enter_context(nc.sbuf_tensor("sq_t", [_P, _FEAT], f32))
        xm2_t = _es.enter_context(nc.sbuf_tensor("xm2_t", [_P, _FEAT], bf16))
        u_t = _es.enter_context(nc.sbuf_tensor("u_t", [_P, _FEAT], bf16))
        junk_t = _es.enter_context(nc.sbuf_tensor("junk_t", [_P, _FEAT], bf16))
        sx_t = _es.enter_context(nc.sbuf_tensor("sx_t", [_P, 2], bf16))
        ones_t = _es.enter_context(nc.sbuf_tensor("ones_t", [_P, 1], bf16))
        res_t = _es.enter_context(nc.sbuf_tensor("res_t", [1, 2], f32))
        acc_t = _es.enter_context(nc.psum_tensor("acc_t", [1, 2], f32))
        lab_sem = _es.enter_context(nc.semaphore("lab_sem"))
        x_sem = _es.enter_context(nc.semaphore("x_sem"))
        c_sem = _es.enter_context(nc.semaphore("c_sem"))
        dve_sem = _es.enter_context(nc.semaphore("dve_sem"))
        m_sem = _es.enter_context(nc.semaphore("m_sem"))
        o_sem = _es.enter_context(nc.semaphore("o_sem"))
        decoy_sem = _es.enter_context(nc.semaphore("decoy_sem"))

        # t0 issues: labels lead the SP HWDGE FIFO, x follows
        nc.sync.dma_start(lab_t.ap(), labels_d).then_inc(lab_sem, 16)
        nc.sync.dma_start(x_t.ap(), x_d).then_inc(x_sem, 16)
        nc.vector.memset(ones_t.ap(), 1.0)
        nc.gpsimd.dma_start(decoy_t.ap(), centers_d[0:1, 0:4]).then_inc(
            decoy_sem, 16
        )

        # gather centers[labels] rows, casting f32 -> bf16 in the SDMA
        nc.gpsimd.wait_ge(lab_sem, 16)
        nc.gpsimd.indirect_dma_start(
            out=c_t.ap(),
            out_offset=None,
            in_=centers_d,
            in_offset=bass.IndirectOffsetOnAxis(ap=lab_t.ap()[:, :1], axis=0),
        ).then_inc(c_sem, 16)
        nc.gpsimd.wait_ge(decoy_sem, 16)

        # pre-gather DVE work, hidden under the gather latency
        nc.vector.wait_ge(x_sem, 16)
        with nc.allow_low_precision("row sums ~1e3, tolerance 2e-2"):
            nc.vector.scalar_tensor_tensor(
                out=sq_t.ap(), in0=x_t.ap(), scalar=1.0, in1=x_t.ap(),
                op0=mybir.AluOpType.mult, op1=mybir.AluOpType.mult,
                accum_out=sx_t.ap()[:, 1:2],
            )
        nc.vector.tensor_scalar_mul(xm2_t.ap(), x_t.ap(), -2.0)
        # post-gather: s1 = rowsum(c*(c-2x)) -> sx[:,0]
        nc.vector.wait_ge(c_sem, 16)
        nc.vector.tensor_tensor(
            out=u_t.ap(), in0=c_t.ap(), in1=xm2_t.ap(), op=mybir.AluOpType.add
        )
        with nc.allow_low_precision("row sums ~1e3, tolerance 2e-2"):
            nc.vector.scalar_tensor_tensor(
                out=junk_t.ap(), in0=u_t.ap(), scalar=0.0, in1=c_t.ap(),
                op0=mybir.AluOpType.add, op1=mybir.AluOpType.mult,
                accum_out=sx_t.ap()[:, 0:1],
            ).then_inc(dve_sem, 1)

        # partition reduction of both columns: acc[1,2] = ones.T @ sx
        nc.tensor.wait_ge(dve_sem, 1)
        nc.tensor.matmul(
            acc_t.ap(), lhsT=ones_t.ap(), rhs=sx_t.ap(), start=True, stop=True
        ).then_inc(m_sem, 1)

        nc.vector.wait_ge(m_sem, 1)
        nc.vector.tensor_copy(out=res_t.ap(), in_=acc_t.ap()).then_inc(dve_sem, 1)

        nc.sync.wait_ge(dve_sem, 2)
        nc.sync.dma_start(out_d, res_t.ap()).then_inc(o_sem, 16)

    nc.compile()
    return nc


def _run(x, labels, centers, trace=False):
    from concourse.bass_utils import run_bass_kernel_spmd
    import ml_dtypes

    if "nc" not in _state:
        _state["nc"] = _build_nc()
    nc = _state["nc"]

    x = np.ascontiguousarray(
        np.asarray(x, dtype=np.float32).astype(ml_dtypes.bfloat16)
    ).reshape(_NCORES, _ROWS, _FEAT)
    lab = np.asarray(labels).astype(np.int32).reshape(_NCORES, _ROWS, 1)
    lab = np.ascontiguousarray(np.repeat(lab, _LABPAD, axis=2))
    cen = np.ascontiguousarray(np.asarray(centers, dtype=np.float32))
    in_maps = [{"x": x[i], "labels": lab[i], "centers": cen} for i in range(_NCORES)]
    res = run_bass_kernel_spmd(nc, in_maps, core_ids=list(range(_NCORES)), trace=trace)
    total = 0.0
    for r in res.results:
        total += float(r["out"][0, 0]) + float(r["out"][0, 1])
    loss = total / _BATCH + (_NCLASSES - 1) * 1e-12
    return np.float32(loss), res


def kernel(x, labels, centers):
    loss, _ = _run(x, labels, centers, trace=False)
    return loss
